# revision 14
# baseline (speedup 1.0000x reference)
"""Trainium2 Bass kernel for nn_Attention_62130996904205.

Full computation (reference):
    q = left @ Wq;  k,v = split(right @ Wkv)
    per head: S = scale * q k^T; S = where(mask, S, -1e7)
    out = (softmax(S) @ v) rearranged @ Wout + bout

Sharding: 8 cores = (batch b in 0..3) x (head-half in 0..1).  Host sums
the two head-half partials per batch and adds bout.

On-chip layout ("S^T scheme"): kv token index n stays on the partition
axis.  PE-array packing (both directions):
  * S matmuls: contraction is DH=64, so the two heads of a pair run
    concurrently in the 128-row array via row tiling
    (tile_position (0,0) / (64,0)).
  * O matmuls: output is 64 rows per head, so the two heads run
    concurrently via column tiling ((0,0) / (0,64)) into one
    (128, MCH) accumulator.
  * softmax denominators: 4 ones-column matmuls (2 heads x 2 nt-halves)
    run concurrently via 4-way column tiling into partitions
    0/32/64/96 of a second accumulator.

exp+mask is split between two engine paths per kv-tile:
  * DVE path: one fused scalar_tensor_tensor computes
        pm_bits = int16(A*s + maskbias)    (A=128/ln2 folded into Wq)
    -- bf16-bitcast Schraudolph fast-exp; masked entries get bias
    B-60A and land at ~1e-24 (an exact-enough zero).  exp+mask+cast in
    a single op.
  * ACT path: scalar-engine Exp (scale=1/A), then the mask multiply is
    fused into the mask's own HBM load: a gpsimd (SWDGE) DMA with
    accum_op=mult reads mask01 from DRAM and multiplies it into the
    exp tile in SBUF.  No vector-engine time at all.
The per-ntp path split load-balances scalar vs vector engines and
limits the Schraudolph share (accuracy).
"""

import numpy as np
import ml_dtypes

import concourse.bass as bass
import concourse.mybir as mybir
import concourse.tile as tile
from concourse import bacc
from concourse.bass_utils import run_bass_kernel_spmd

BF16 = ml_dtypes.bfloat16
FP32 = np.float32

# Schraudolph fast-exp constants (bf16 bitcast): bits = int16(A*s + B)
SCHR_A = 128.0 / np.log(2.0)          # 184.6627
SCHR_B = 16249.0                      # calibrated (round-to-nearest on HW)
SCHR_BMASK = 5113.0                   # B - 11136: masked -> ~1e-25 (== 0)
SCHR_MADD = -11136                    # SWDGE bits-domain mask subtrahend

# per-ntp engine-path assignment, balanced for ACT/DVE load:
#   D: fused DVE scalar_tensor_tensor Schraudolph (exp+mask+cast, 1 op)
#   W: ACT Schraudolph copy -> int16, mask added in bits-domain by the
#      mask's own SWDGE DMA (accum_op=add) -- no DVE time at all
#   E: ACT true Exp -> bf16, then 2x-rate DVE mask multiply
PATH = {1: "D", 4: "D", 7: "D", 9: "D", 12: "D", 15: "D",
        2: "W", 6: "W", 10: "W", 14: "W",
        0: "E", 3: "E", 5: "E", 8: "E", 11: "E", 13: "E"}

# set by test harness to enable NTFF tracing
TRACE = False
LAST_RESULTS = None


def build_core(M=1024, N=4096, DQ=512, H=4, DH=64):
    """Build the per-core Bass program. Every core runs this same program
    on its own shard (SPMD)."""
    dt = mybir.dt
    f32, bf16, i16 = dt.float32, dt.bfloat16, dt.int16
    D = H * DH            # head features handled by this core
    KT = DQ // 128        # contraction tiles for the projections
    NT = N // 128         # kv-token tiles
    NTP = NT // 2         # processed in pairs of two 128-tiles
    MCH = min(512, M)     # m-chunk (PSUM free width per O accumulator)
    NMC = M // MCH
    SW = 2 * MCH          # S-psum width: one nt-pair worth of logits
    KT2 = D // 128        # contraction tiles for the output projection
    VW = H * DH           # v-projection free width

    assert M % MCH == 0 and N % 256 == 0 and DQ % 128 == 0 and D % 128 == 0

    nc = bacc.Bacc("TRN2", target_bir_lowering=False, debug=False)

    leftT = nc.dram_tensor("leftT", [DQ, M], bf16, kind="ExternalInput")
    rightT = nc.dram_tensor("rightT", [DQ, N], bf16, kind="ExternalInput")
    maskbias = nc.dram_tensor("maskbias", [N, M], i16, kind="ExternalInput")
    maskadd = nc.dram_tensor("maskadd", [N, M], i16, kind="ExternalInput")
    mask01 = nc.dram_tensor("mask01", [N, M], bf16, kind="ExternalInput")
    wq = nc.dram_tensor("wq", [DQ, D], bf16, kind="ExternalInput")
    wk = nc.dram_tensor("wk", [DQ, D], bf16, kind="ExternalInput")
    wv = nc.dram_tensor("wv", [DQ, D], bf16, kind="ExternalInput")
    wout = nc.dram_tensor("wout", [D, DQ], bf16, kind="ExternalInput")
    out_p = nc.dram_tensor("out_p", [M, DQ], f32, kind="ExternalOutput")

    EXP = mybir.ActivationFunctionType.Exp
    MUL = mybir.AluOpType.mult
    ADD = mybir.AluOpType.add

    with tile.TileContext(nc) as tc:
        with (
            tc.tile_pool(name="sing", bufs=1) as sing,
            tc.tile_pool(name="spool", bufs=3, space="PSUM") as spool,
            tc.tile_pool(name="opool", bufs=1, space="PSUM") as opool,
            tc.tile_pool(
                name="mpool",
                bufs=sum(1 for v in PATH.values() if v != "W") + 2,
            ) as mpool,
            tc.tile_pool(name="ppool", bufs=8) as ppool,
            tc.tile_pool(name="smallp", bufs=2) as smallp,
            tc.tile_pool(name="outp", bufs=3) as outp,
        ):
            # ---- weight + activation loads -------------------------------
            wq_sb = sing.tile([128, KT, D], bf16, tag="wq")
            nc.sync.dma_start(out=wq_sb, in_=wq.rearrange("(kt p) d -> p kt d", p=128))
            leftT_sb = []
            for kt in range(KT):
                t = sing.tile([128, M], bf16, tag=f"leftT{kt}", name=f"leftT{kt}")
                nc.sync.dma_start(out=t, in_=leftT[kt * 128 : (kt + 1) * 128, :])
                leftT_sb.append(t)
            wk_sb = sing.tile([128, KT, D], bf16, tag="wk")
            nc.scalar.dma_start(out=wk_sb, in_=wk.rearrange("(kt p) d -> p kt d", p=128))
            wv_sb = sing.tile([128, KT, D], bf16, tag="wv")
            nc.scalar.dma_start(out=wv_sb, in_=wv.rearrange("(kt p) d -> p kt d", p=128))
            RCH = min(N, 1024)
            rightT_sb = [
                sing.tile([128, N], bf16, tag=f"rightT{kt}", name=f"rightT{kt}")
                for kt in range(KT)
            ]
            for c in range(N // RCH):
                for kt in range(KT):
                    eng = nc.scalar if (c * KT + kt) % 2 == 0 else nc.sync
                    eng.dma_start(
                        out=rightT_sb[kt][:, c * RCH : (c + 1) * RCH],
                        in_=rightT[
                            kt * 128 : (kt + 1) * 128, c * RCH : (c + 1) * RCH
                        ],
                    )
            wout_sb = sing.tile([128, KT2, DQ], bf16, tag="wout")
            nc.sync.dma_start(
                out=wout_sb, in_=wout.rearrange("(kt p) d -> p kt d", p=128)
            )

            # ---- projections ---------------------------------------------
            # q^T / k^T as paired-head tiles: head 2*hp in partitions 0..63,
            # head 2*hp+1 in partitions 64..127 (row-tiled S matmuls).
            qT2 = [sing.tile([128, M], bf16, tag=f"qT{h}", name=f"qT{h}") for h in range(H // 2)]
            kT2 = [sing.tile([128, N], bf16, tag=f"kT{h}", name=f"kT{h}") for h in range(H // 2)]
            u_sb = [sing.tile([128, M], bf16, tag=f"u{p}", name=f"u{p}") for p in range(KT2)]
            v_sb = sing.tile([128, NT, H, DH], bf16, tag="v")
            ones1 = sing.tile([128, 1], bf16, tag="ones1")
            nc.vector.memset(ones1, 1.0)

            for t2 in range(H // 2):
                ps = spool.tile([128, SW], f32, tag="s")
                w512 = min(512, M)
                for mh in range(M // w512):
                    for kt in range(KT):
                        nc.tensor.matmul(
                            ps[:, mh * w512 : (mh + 1) * w512],
                            lhsT=wq_sb[:, kt, t2 * 128 : (t2 + 1) * 128],
                            rhs=leftT_sb[kt][:, mh * w512 : (mh + 1) * w512],
                            start=(kt == 0),
                            stop=(kt == KT - 1),
                        )
                nc.scalar.copy(out=qT2[t2][:, :], in_=ps[:, 0:M])

            CW = min(SW, N)
            NKC = N // CW

            def k_chunk(t2, cp):
                """k-projection for one head pair, one N-chunk."""
                ps = spool.tile([128, SW], f32, tag="s", name="kps")
                w512 = min(512, CW)
                for half in range(CW // w512):
                    for kt in range(KT):
                        nc.tensor.matmul(
                            ps[:, half * w512 : (half + 1) * w512],
                            lhsT=wk_sb[:, kt, t2 * 128 : (t2 + 1) * 128],
                            rhs=rightT_sb[kt][
                                :, cp * CW + half * w512 : cp * CW + (half + 1) * w512
                            ],
                            start=(kt == 0),
                            stop=(kt == KT - 1),
                        )
                nc.scalar.copy(
                    out=kT2[t2][:, cp * CW : (cp + 1) * CW], in_=ps[:, 0:CW]
                )

            def v_nt4(g):
                """v-projection for four kv-token tiles (one evac copy)."""
                ps = spool.tile([128, SW], f32, tag="s", name="vps")
                for j in range(SW // VW):
                    nt = g * (SW // VW) + j
                    for kt in range(KT):
                        nc.tensor.matmul(
                            ps[:, j * VW : (j + 1) * VW],
                            lhsT=rightT_sb[kt][:, nt * 128 : (nt + 1) * 128],
                            rhs=wv_sb[:, kt, :],
                            start=(kt == 0),
                            stop=(kt == KT - 1),
                        )
                nvt = SW // VW
                nc.scalar.copy(
                    out=v_sb[:, g * nvt : (g + 1) * nvt, :, :],
                    in_=ps[:, 0 : nvt * VW],
                )

            def outproj_mt(mt):
                """output projection for one 128-row m-slice."""
                ps = spool.tile([128, SW], f32, tag="s", name="ops")
                for p2 in range(KT2):
                    nc.tensor.matmul(
                        ps[:, 0:DQ],
                        lhsT=u_sb[p2][:, mt * 128 : (mt + 1) * 128],
                        rhs=wout_sb[:, p2, :],
                        start=(p2 == 0),
                        stop=(p2 == KT2 - 1),
                    )
                ob = outp.tile([128, DQ], f32, tag="ob")
                nc.scalar.copy(out=ob, in_=ps[:, 0:DQ])
                nc.sync.dma_start(out=out_p[mt * 128 : (mt + 1) * 128, :], in_=ob)

            # upfront prerequisites for attention (mc0, hp0); the rest is
            # paced into the attention stream (2 pops per ntp).
            NVG = SW // VW
            k_chunk(0, 0)
            v_nt4(0)
            events = []
            for g in range(1, NT // NVG):
                events.append((max(0, 4 * g - 8), lambda g=g: v_nt4(g)))
            for cp in range(1, NKC):
                events.append((max(0, 8 * cp - 6), lambda cp=cp: k_chunk(0, cp)))
            if H > 2:
                for cp in range(NKC):
                    events.append((18 + 8 * cp, lambda cp=cp: k_chunk(1, cp)))
            events.sort(key=lambda e: e[0])
            deferred = [fn for _, fn in events]
            lazy = []  # deadline-free filler (out-projections)

            # ---- attention ----------------------------------------------
            DEPTH = 2  # oq entries are whole ntps now (2 tiles each)
            for mc in range(NMC):
                msks = {}
                for ntp in range(NTP):
                    path = PATH[ntp % 16]
                    if path == "W":
                        continue  # mask arrives via SWDGE accumulate
                    src = maskbias if path == "D" else mask01
                    mdt = i16 if path == "D" else bf16
                    msk = mpool.tile([128, SW], mdt, tag="msk", name=f"msk{ntp}")
                    nc.sync.dma_start(
                        out=msk,
                        in_=src[
                            ntp * 256 : (ntp + 1) * 256, mc * MCH : (mc + 1) * MCH
                        ].rearrange("(a p) f -> p a f", p=128),
                    )
                    msks[ntp] = msk
                for hp in range(H // 2):
                    o_ps = opool.tile([128, MCH], f32, tag="o", name="o")
                    dn_ps = opool.tile([128, MCH], f32, tag="dn", name="dn")
                    oq = []          # deferred (pmA, pmB, ntp)
                    state = {"started": False}

                    def make_flush(o_ps, dn_ps, oq, state, hp):
                        def flush_one():
                            pms, ntp_ = oq.pop(0)
                            first = not state["started"]
                            state["started"] = True
                            last = ntp_ == NTP - 1
                            for half in range(2):
                                nt = 2 * ntp_ + half
                                for i in range(2):
                                    nc.tensor.matmul(
                                        o_ps[64 * i : 64 * i + 64, :],
                                        lhsT=v_sb[:, nt, 2 * hp + i, :],
                                        rhs=pms[i][:, half * MCH : (half + 1) * MCH],
                                        start=first and half == 0,
                                        stop=last and half == 1,
                                        tile_position=(0, 64 * i),
                                    )
                            for c in range(4):
                                i, half = c >> 1, c & 1
                                nc.tensor.matmul(
                                    dn_ps[32 * c : 32 * c + 1, :],
                                    lhsT=ones1,
                                    rhs=pms[i][:, half * MCH : (half + 1) * MCH],
                                    start=first,
                                    stop=last,
                                    tile_position=(0, 32 * c),
                                )
                        return flush_one

                    flush_one = make_flush(o_ps, dn_ps, oq, state, hp)

                    for ntp in range(NTP):
                        # two heads' S^T concurrently via PE row tiling
                        s_ab = [spool.tile([128, SW], f32, tag="s", name=f"s{i}")
                                for i in range(2)]
                        for half in range(2):
                            nt = 2 * ntp + half
                            for i in range(2):
                                lo = 64 * i
                                nc.tensor.matmul(
                                    s_ab[i][:, half * MCH : (half + 1) * MCH],
                                    lhsT=kT2[hp][lo : lo + 64, nt * 128 : (nt + 1) * 128],
                                    rhs=qT2[hp][lo : lo + 64, mc * MCH : (mc + 1) * MCH],
                                    start=True,
                                    stop=True,
                                    tile_position=(lo, 0),
                                )
                        path = PATH[ntp % 16]
                        pms = []
                        for i in range(2):
                            pm = ppool.tile([128, SW], bf16, tag="p")
                            if path == "D":
                                # fused Schraudolph: exp+mask+cast in one op
                                nc.vector.scalar_tensor_tensor(
                                    out=pm.bitcast(i16),
                                    in0=s_ab[i][:, :],
                                    scalar=1.0,
                                    in1=msks[ntp][:, :],
                                    op0=MUL,
                                    op1=ADD,
                                )
                            elif path == "W":
                                # ACT Schraudolph; mask added by its own DMA
                                nc.scalar.activation(
                                    pm.bitcast(i16), s_ab[i],
                                    mybir.ActivationFunctionType.Copy,
                                    bias=SCHR_B, scale=1.0,
                                )
                                nc.gpsimd.dma_start(
                                    out=pm.bitcast(i16),
                                    in_=maskadd[
                                        ntp * 256 : (ntp + 1) * 256,
                                        mc * MCH : (mc + 1) * MCH,
                                    ].rearrange("(a p) f -> p a f", p=128),
                                    accum_op=ADD,
                                )
                            else:
                                p_sb = ppool.tile([128, SW], bf16, tag="pe")
                                nc.scalar.activation(
                                    p_sb, s_ab[i], EXP, scale=float(1.0 / SCHR_A)
                                )
                                nc.vector.tensor_mul(pm, p_sb, msks[ntp])
                            pms.append(pm)
                            if deferred:
                                deferred.pop(0)()
                            elif lazy and i == 0:
                                lazy.pop(0)()
                        oq.append((pms, ntp))
                        if len(oq) > DEPTH:
                            flush_one()
                    while oq:
                        flush_one()
                    if mc == 0 and hp == 0:
                        while deferred:
                            deferred.pop(0)()
                    # normalize: U^T = O^T * (1/d) broadcast over partitions
                    for i in range(2):
                        h = 2 * hp + i
                        dh = smallp.tile([1, MCH], f32, tag="dh", name=f"dh{i}")
                        nc.scalar.copy(out=dh, in_=dn_ps[64 * i : 64 * i + 1, :])
                        dsum = smallp.tile([1, MCH], f32, tag="ds", name=f"ds{i}")
                        nc.vector.scalar_tensor_tensor(
                            out=dsum,
                            in0=dn_ps[64 * i + 32 : 64 * i + 33, :],
                            scalar=1.0,
                            in1=dh,
                            op0=MUL,
                            op1=ADD,
                        )
                        rd = smallp.tile([1, MCH], f32, tag="rd", name=f"rd{i}")
                        nc.vector.reciprocal_approx_fast(out=rd, in_=dsum)
                        bd = smallp.tile([64, MCH], f32, tag="bd", name=f"bd{i}")
                        nc.gpsimd.partition_broadcast(bd, rd)
                        nc.vector.tensor_mul(
                            u_sb[h // 2][
                                (h % 2) * 64 : (h % 2) * 64 + 64,
                                mc * MCH : (mc + 1) * MCH,
                            ],
                            o_ps[64 * i : 64 * i + 64, :],
                            bd,
                        )
                for mt in range(mc * MCH // 128, (mc + 1) * MCH // 128):
                    lazy.append(lambda mt=mt: outproj_mt(mt))

            while deferred:
                deferred.pop(0)()
            while lazy:
                lazy.pop(0)()

    nc.finalize()
    return nc


_NC_CACHE = {}


def _get_nc(key=(1024, 4096, 512, 4, 64)):
    if key not in _NC_CACHE:
        _NC_CACHE[key] = build_core(*key)
    return _NC_CACHE[key]


def kernel(left, right, mask, Wq, Wkv, Wout, bout):
    """Full-input entry point: shards across 8 neuron cores, returns the
    full (B, M, DQ) output."""
    global LAST_RESULTS
    B, M, DQmat = left.shape
    _, N, DC = right.shape
    H, DH = 8, 64
    D = H * DH
    Hc = H // 2          # heads per core
    scale = DH ** -0.5

    left = np.asarray(left, dtype=np.float32)
    right = np.asarray(right, dtype=np.float32)
    Wq = np.asarray(Wq, dtype=np.float32)
    Wkv = np.asarray(Wkv, dtype=np.float32)
    Wout = np.asarray(Wout, dtype=np.float32)
    bout = np.asarray(bout, dtype=np.float32)

    # 1/sqrt(DH) and the Schraudolph log2-scale are folded into Wq
    Wqs = (Wq * (scale * SCHR_A)).astype(BF16)  # (DQ, D)
    Wk = Wkv[:, :D].astype(BF16)               # (DC, D)
    Wv = Wkv[:, D:].astype(BF16)               # (DC, D)
    WoutB = Wout.astype(BF16)                  # (D, DQ)

    leftT = np.ascontiguousarray(left.transpose(0, 2, 1)).astype(BF16)    # (B, DQ, M)
    rightT = np.ascontiguousarray(right.transpose(0, 2, 1)).astype(BF16)  # (B, DC, N)
    maskT = np.ascontiguousarray(mask.transpose(0, 2, 1))                 # (B, N, M)
    mb = np.where(maskT, np.int16(int(SCHR_B)), np.int16(int(SCHR_BMASK)))
    ma = np.where(maskT, np.int16(0), np.int16(SCHR_MADD))
    m01 = maskT.astype(BF16)

    nc = _get_nc((M, N, DQmat, Hc, DH))

    in_maps = []
    for core in range(8):
        b, hh = divmod(core, 2)
        hs = slice(hh * Hc * DH, (hh + 1) * Hc * DH)
        in_maps.append(
            {
                "leftT": leftT[b],
                "rightT": rightT[b],
                "maskbias": mb[b],
                "maskadd": ma[b],
                "mask01": m01[b],
                "wq": np.ascontiguousarray(Wqs[:, hs]),
                "wk": np.ascontiguousarray(Wk[:, hs]),
                "wv": np.ascontiguousarray(Wv[:, hs]),
                "wout": np.ascontiguousarray(WoutB[hs, :]),
            }
        )

    tmpdir = None
    if TRACE:
        import shutil

        shutil.rmtree("/tmp/attn_trace", ignore_errors=True)
        tmpdir = "/tmp/attn_trace"
    res = run_bass_kernel_spmd(nc, in_maps, list(range(8)), trace=TRACE, tmpdir=tmpdir)
    LAST_RESULTS = res

    out = np.zeros((B, M, DQmat), np.float32)
    for core in range(8):
        out[core // 2] += res.results[core]["out_p"]
    out += bout[None, None, :]
    return out


# revision 15
# speedup vs baseline: 1.2196x; 1.2196x over previous
"""Trainium2 Bass kernel for nn_Attention_62130996904205.

Full computation (reference):
    q = left @ Wq;  k,v = split(right @ Wkv)
    per head: S = scale * q k^T; S = where(mask, S, -1e7)
    out = (softmax(S) @ v) rearranged @ Wout + bout

Sharding: 8 cores = (batch b in 0..3) x (head-half in 0..1).  Host sums
the two head-half partials per batch and adds bout.

On-chip layout ("S^T scheme"): kv token index n stays on the partition
axis.  PE-array packing (both directions):
  * S matmuls: contraction is DH=64, so the two heads of a pair run
    concurrently in the 128-row array via row tiling
    (tile_position (0,0) / (64,0)).
  * O matmuls: output is 64 rows per head, so the two heads run
    concurrently via column tiling ((0,0) / (0,64)) into one
    (128, MCH) accumulator.
  * softmax denominators: 4 ones-column matmuls (2 heads x 2 nt-halves)
    run concurrently via 4-way column tiling into partitions
    0/32/64/96 of a second accumulator.

exp+mask is split between two engine paths per kv-tile:
  * DVE path: one fused scalar_tensor_tensor computes
        pm_bits = int16(A*s + maskbias)    (A=128/ln2 folded into Wq)
    -- bf16-bitcast Schraudolph fast-exp; masked entries get bias
    B-60A and land at ~1e-24 (an exact-enough zero).  exp+mask+cast in
    a single op.
  * ACT path: scalar-engine Exp (scale=1/A), then the mask multiply is
    fused into the mask's own HBM load: a gpsimd (SWDGE) DMA with
    accum_op=mult reads mask01 from DRAM and multiplies it into the
    exp tile in SBUF.  No vector-engine time at all.
The per-ntp path split load-balances scalar vs vector engines and
limits the Schraudolph share (accuracy).
"""

import numpy as np
import ml_dtypes

import concourse.bass as bass
import concourse.mybir as mybir
import concourse.tile as tile
from concourse import bacc
from concourse.bass_utils import run_bass_kernel_spmd

BF16 = ml_dtypes.bfloat16
FP32 = np.float32

# Schraudolph fast-exp constants (bf16 bitcast): bits = int16(A*s + B)
SCHR_A = 128.0 / np.log(2.0)          # 184.6627
SCHR_B = 16249.0                      # calibrated (round-to-nearest on HW)
SCHR_BMASK = 5113.0                   # B - 11136: masked -> ~1e-25 (== 0)
SCHR_MADD = -11136                    # SWDGE bits-domain mask subtrahend

# per-ntp engine-path assignment, balanced for ACT/DVE load:
#   D: fused DVE scalar_tensor_tensor Schraudolph (exp+mask+cast, 1 op)
#   W: ACT Schraudolph copy -> int16, mask added in bits-domain by the
#      mask's own SWDGE DMA (accum_op=add) -- no DVE time at all
#   E: ACT true Exp -> bf16, then 2x-rate DVE mask multiply
PATH = {i: ("D" if i in (1, 4, 7, 10, 13) else "E") for i in range(16)}

# set by test harness to enable NTFF tracing
TRACE = False
LAST_RESULTS = None


def build_core(M=1024, N=4096, DQ=512, H=4, DH=64):
    """Build the per-core Bass program. Every core runs this same program
    on its own shard (SPMD)."""
    dt = mybir.dt
    f32, bf16, i16 = dt.float32, dt.bfloat16, dt.int16
    D = H * DH            # head features handled by this core
    KT = DQ // 128        # contraction tiles for the projections
    NT = N // 128         # kv-token tiles
    NTP = NT // 2         # processed in pairs of two 128-tiles
    MCH = min(512, M)     # m-chunk (PSUM free width per O accumulator)
    NMC = M // MCH
    SW = 2 * MCH          # S-psum width: one nt-pair worth of logits
    KT2 = D // 128        # contraction tiles for the output projection
    VW = H * DH           # v-projection free width

    assert M % MCH == 0 and N % 256 == 0 and DQ % 128 == 0 and D % 128 == 0

    nc = bacc.Bacc("TRN2", target_bir_lowering=False, debug=False)

    leftT = nc.dram_tensor("leftT", [DQ, M], bf16, kind="ExternalInput")
    rightT = nc.dram_tensor("rightT", [DQ, N], bf16, kind="ExternalInput")
    maskbias = nc.dram_tensor("maskbias", [N, M], i16, kind="ExternalInput")
    maskadd = nc.dram_tensor("maskadd", [N, M], i16, kind="ExternalInput")
    mask01 = nc.dram_tensor("mask01", [N, M], bf16, kind="ExternalInput")
    wq = nc.dram_tensor("wq", [DQ, D], bf16, kind="ExternalInput")
    wk = nc.dram_tensor("wk", [DQ, D], bf16, kind="ExternalInput")
    wv = nc.dram_tensor("wv", [DQ, D], bf16, kind="ExternalInput")
    wout = nc.dram_tensor("wout", [D, DQ], bf16, kind="ExternalInput")
    out_p = nc.dram_tensor("out_p", [M, DQ], f32, kind="ExternalOutput")

    EXP = mybir.ActivationFunctionType.Exp
    MUL = mybir.AluOpType.mult
    ADD = mybir.AluOpType.add

    with tile.TileContext(nc) as tc:
        with (
            tc.tile_pool(name="sing", bufs=1) as sing,
            tc.tile_pool(name="spool", bufs=3, space="PSUM") as spool,
            tc.tile_pool(name="opool", bufs=1, space="PSUM") as opool,
            tc.tile_pool(
                name="mpool",
                bufs=sum(1 for v in PATH.values() if v != "W") + 2,
            ) as mpool,
            tc.tile_pool(name="ppool", bufs=8) as ppool,
            tc.tile_pool(name="smallp", bufs=2) as smallp,
            tc.tile_pool(name="outp", bufs=3) as outp,
        ):
            # ---- weight + activation loads -------------------------------
            wq_sb = sing.tile([128, KT, D], bf16, tag="wq")
            nc.sync.dma_start(out=wq_sb, in_=wq.rearrange("(kt p) d -> p kt d", p=128))
            leftT_sb = []
            for kt in range(KT):
                t = sing.tile([128, M], bf16, tag=f"leftT{kt}", name=f"leftT{kt}")
                nc.sync.dma_start(out=t, in_=leftT[kt * 128 : (kt + 1) * 128, :])
                leftT_sb.append(t)
            wk_sb = sing.tile([128, KT, D], bf16, tag="wk")
            nc.scalar.dma_start(out=wk_sb, in_=wk.rearrange("(kt p) d -> p kt d", p=128))
            wv_sb = sing.tile([128, KT, D], bf16, tag="wv")
            nc.scalar.dma_start(out=wv_sb, in_=wv.rearrange("(kt p) d -> p kt d", p=128))
            RCH = min(N, 1024)
            rightT_sb = [
                sing.tile([128, N], bf16, tag=f"rightT{kt}", name=f"rightT{kt}")
                for kt in range(KT)
            ]
            for c in range(N // RCH):
                for kt in range(KT):
                    eng = nc.scalar if (c * KT + kt) % 2 == 0 else nc.sync
                    eng.dma_start(
                        out=rightT_sb[kt][:, c * RCH : (c + 1) * RCH],
                        in_=rightT[
                            kt * 128 : (kt + 1) * 128, c * RCH : (c + 1) * RCH
                        ],
                    )
            wout_sb = sing.tile([128, KT2, DQ], bf16, tag="wout")
            nc.sync.dma_start(
                out=wout_sb, in_=wout.rearrange("(kt p) d -> p kt d", p=128)
            )

            # ---- projections ---------------------------------------------
            # q^T / k^T as paired-head tiles: head 2*hp in partitions 0..63,
            # head 2*hp+1 in partitions 64..127 (row-tiled S matmuls).
            qT2 = [sing.tile([128, M], bf16, tag=f"qT{h}", name=f"qT{h}") for h in range(H // 2)]
            kT2 = [sing.tile([128, N], bf16, tag=f"kT{h}", name=f"kT{h}") for h in range(H // 2)]
            u_sb = [sing.tile([128, M], bf16, tag=f"u{p}", name=f"u{p}") for p in range(KT2)]
            v_sb = sing.tile([128, NT, H, DH], bf16, tag="v")
            ones1 = sing.tile([128, 1], bf16, tag="ones1")
            nc.vector.memset(ones1, 1.0)

            for t2 in range(H // 2):
                ps = spool.tile([128, SW], f32, tag="s")
                w512 = min(512, M)
                for mh in range(M // w512):
                    for kt in range(KT):
                        nc.tensor.matmul(
                            ps[:, mh * w512 : (mh + 1) * w512],
                            lhsT=wq_sb[:, kt, t2 * 128 : (t2 + 1) * 128],
                            rhs=leftT_sb[kt][:, mh * w512 : (mh + 1) * w512],
                            start=(kt == 0),
                            stop=(kt == KT - 1),
                        )
                nc.scalar.copy(out=qT2[t2][:, :], in_=ps[:, 0:M])

            CW = min(SW, N)
            NKC = N // CW

            def k_chunk(t2, cp):
                """k-projection for one head pair, one N-chunk."""
                ps = spool.tile([128, SW], f32, tag="s", name="kps")
                w512 = min(512, CW)
                for half in range(CW // w512):
                    for kt in range(KT):
                        nc.tensor.matmul(
                            ps[:, half * w512 : (half + 1) * w512],
                            lhsT=wk_sb[:, kt, t2 * 128 : (t2 + 1) * 128],
                            rhs=rightT_sb[kt][
                                :, cp * CW + half * w512 : cp * CW + (half + 1) * w512
                            ],
                            start=(kt == 0),
                            stop=(kt == KT - 1),
                        )
                nc.scalar.copy(
                    out=kT2[t2][:, cp * CW : (cp + 1) * CW], in_=ps[:, 0:CW]
                )

            def v_nt4(g):
                """v-projection for four kv-token tiles (one evac copy)."""
                ps = spool.tile([128, SW], f32, tag="s", name="vps")
                for j in range(SW // VW):
                    nt = g * (SW // VW) + j
                    for kt in range(KT):
                        nc.tensor.matmul(
                            ps[:, j * VW : (j + 1) * VW],
                            lhsT=rightT_sb[kt][:, nt * 128 : (nt + 1) * 128],
                            rhs=wv_sb[:, kt, :],
                            start=(kt == 0),
                            stop=(kt == KT - 1),
                        )
                nvt = SW // VW
                nc.scalar.copy(
                    out=v_sb[:, g * nvt : (g + 1) * nvt, :, :],
                    in_=ps[:, 0 : nvt * VW],
                )

            def outproj_mt(mt):
                """output projection for one 128-row m-slice."""
                ps = spool.tile([128, SW], f32, tag="s", name="ops")
                for p2 in range(KT2):
                    nc.tensor.matmul(
                        ps[:, 0:DQ],
                        lhsT=u_sb[p2][:, mt * 128 : (mt + 1) * 128],
                        rhs=wout_sb[:, p2, :],
                        start=(p2 == 0),
                        stop=(p2 == KT2 - 1),
                    )
                ob = outp.tile([128, DQ], f32, tag="ob")
                nc.scalar.copy(out=ob, in_=ps[:, 0:DQ])
                nc.sync.dma_start(out=out_p[mt * 128 : (mt + 1) * 128, :], in_=ob)

            # upfront prerequisites for attention (mc0, hp0); the rest is
            # paced into the attention stream (2 pops per ntp).
            NVG = SW // VW
            k_chunk(0, 0)
            v_nt4(0)
            events = []
            for g in range(1, NT // NVG):
                events.append((max(0, 4 * g - 8), lambda g=g: v_nt4(g)))
            for cp in range(1, NKC):
                events.append((max(0, 8 * cp - 6), lambda cp=cp: k_chunk(0, cp)))
            if H > 2:
                for cp in range(NKC):
                    events.append((18 + 8 * cp, lambda cp=cp: k_chunk(1, cp)))
            events.sort(key=lambda e: e[0])
            deferred = [fn for _, fn in events]
            lazy = []  # deadline-free filler (out-projections)

            # ---- attention ----------------------------------------------
            DEPTH = 2  # oq entries are whole ntps now (2 tiles each)
            for mc in range(NMC):
                msks = {}
                for ntp in range(NTP):
                    path = PATH[ntp % 16]
                    if path == "W":
                        continue  # mask arrives via SWDGE accumulate
                    src = maskbias if path == "D" else mask01
                    mdt = i16 if path == "D" else bf16
                    msk = mpool.tile([128, SW], mdt, tag="msk", name=f"msk{ntp}")
                    nc.sync.dma_start(
                        out=msk,
                        in_=src[
                            ntp * 256 : (ntp + 1) * 256, mc * MCH : (mc + 1) * MCH
                        ].rearrange("(a p) f -> p a f", p=128),
                    )
                    msks[ntp] = msk
                for hp in range(H // 2):
                    o_ps = opool.tile([128, MCH], f32, tag="o", name="o")
                    dn_ps = opool.tile([128, MCH], f32, tag="dn", name="dn")
                    oq = []          # deferred (pmA, pmB, ntp)
                    state = {"started": False}

                    def make_flush(o_ps, dn_ps, oq, state, hp):
                        def flush_one():
                            pms, ntp_ = oq.pop(0)
                            first = not state["started"]
                            state["started"] = True
                            last = ntp_ == NTP - 1
                            for half in range(2):
                                nt = 2 * ntp_ + half
                                for i in range(2):
                                    nc.tensor.matmul(
                                        o_ps[64 * i : 64 * i + 64, :],
                                        lhsT=v_sb[:, nt, 2 * hp + i, :],
                                        rhs=pms[i][:, half * MCH : (half + 1) * MCH],
                                        start=first and half == 0,
                                        stop=last and half == 1,
                                        tile_position=(0, 64 * i),
                                    )
                            for c in range(4):
                                i, half = c >> 1, c & 1
                                nc.tensor.matmul(
                                    dn_ps[32 * c : 32 * c + 1, :],
                                    lhsT=ones1,
                                    rhs=pms[i][:, half * MCH : (half + 1) * MCH],
                                    start=first,
                                    stop=last,
                                    tile_position=(0, 32 * c),
                                )
                        return flush_one

                    flush_one = make_flush(o_ps, dn_ps, oq, state, hp)

                    for ntp in range(NTP):
                        # two heads' S^T concurrently via PE row tiling
                        s_ab = [spool.tile([128, SW], f32, tag="s", name=f"s{i}")
                                for i in range(2)]
                        for half in range(2):
                            nt = 2 * ntp + half
                            for i in range(2):
                                lo = 64 * i
                                nc.tensor.matmul(
                                    s_ab[i][:, half * MCH : (half + 1) * MCH],
                                    lhsT=kT2[hp][lo : lo + 64, nt * 128 : (nt + 1) * 128],
                                    rhs=qT2[hp][lo : lo + 64, mc * MCH : (mc + 1) * MCH],
                                    start=True,
                                    stop=True,
                                    tile_position=(lo, 0),
                                )
                        path = PATH[ntp % 16]
                        pms = []
                        for i in range(2):
                            pm = ppool.tile([128, SW], bf16, tag="p")
                            if path == "D":
                                # fused Schraudolph: exp+mask+cast in one op
                                nc.vector.scalar_tensor_tensor(
                                    out=pm.bitcast(i16),
                                    in0=s_ab[i][:, :],
                                    scalar=1.0,
                                    in1=msks[ntp][:, :],
                                    op0=MUL,
                                    op1=ADD,
                                )
                            elif path == "W":
                                # ACT Schraudolph; mask added by its own DMA
                                nc.scalar.activation(
                                    pm.bitcast(i16), s_ab[i],
                                    mybir.ActivationFunctionType.Copy,
                                    bias=SCHR_B, scale=1.0,
                                )
                                nc.gpsimd.dma_start(
                                    out=pm.bitcast(i16),
                                    in_=maskadd[
                                        ntp * 256 : (ntp + 1) * 256,
                                        mc * MCH : (mc + 1) * MCH,
                                    ].rearrange("(a p) f -> p a f", p=128),
                                    accum_op=ADD,
                                )
                            else:
                                p_sb = ppool.tile([128, SW], bf16, tag="pe")
                                nc.scalar.activation(
                                    p_sb, s_ab[i], EXP, scale=float(1.0 / SCHR_A)
                                )
                                nc.vector.tensor_mul(pm, p_sb, msks[ntp])
                            pms.append(pm)
                            if deferred:
                                deferred.pop(0)()
                            elif lazy and i == 0:
                                lazy.pop(0)()
                        oq.append((pms, ntp))
                        if len(oq) > DEPTH:
                            flush_one()
                    while oq:
                        flush_one()
                    if mc == 0 and hp == 0:
                        while deferred:
                            deferred.pop(0)()
                    # normalize: U^T = O^T * (1/d) broadcast over partitions
                    for i in range(2):
                        h = 2 * hp + i
                        dh = smallp.tile([1, MCH], f32, tag="dh", name=f"dh{i}")
                        nc.scalar.copy(out=dh, in_=dn_ps[64 * i : 64 * i + 1, :])
                        dsum = smallp.tile([1, MCH], f32, tag="ds", name=f"ds{i}")
                        nc.vector.scalar_tensor_tensor(
                            out=dsum,
                            in0=dn_ps[64 * i + 32 : 64 * i + 33, :],
                            scalar=1.0,
                            in1=dh,
                            op0=MUL,
                            op1=ADD,
                        )
                        rd = smallp.tile([1, MCH], f32, tag="rd", name=f"rd{i}")
                        nc.vector.reciprocal_approx_fast(out=rd, in_=dsum)
                        bd = smallp.tile([64, MCH], f32, tag="bd", name=f"bd{i}")
                        nc.gpsimd.partition_broadcast(bd, rd)
                        nc.vector.tensor_mul(
                            u_sb[h // 2][
                                (h % 2) * 64 : (h % 2) * 64 + 64,
                                mc * MCH : (mc + 1) * MCH,
                            ],
                            o_ps[64 * i : 64 * i + 64, :],
                            bd,
                        )
                for mt in range(mc * MCH // 128, (mc + 1) * MCH // 128):
                    lazy.append(lambda mt=mt: outproj_mt(mt))

            while deferred:
                deferred.pop(0)()
            while lazy:
                lazy.pop(0)()

    nc.finalize()
    return nc


_NC_CACHE = {}


def _get_nc(key=(1024, 4096, 512, 4, 64)):
    if key not in _NC_CACHE:
        _NC_CACHE[key] = build_core(*key)
    return _NC_CACHE[key]


def kernel(left, right, mask, Wq, Wkv, Wout, bout):
    """Full-input entry point: shards across 8 neuron cores, returns the
    full (B, M, DQ) output."""
    global LAST_RESULTS
    B, M, DQmat = left.shape
    _, N, DC = right.shape
    H, DH = 8, 64
    D = H * DH
    Hc = H // 2          # heads per core
    scale = DH ** -0.5

    left = np.asarray(left, dtype=np.float32)
    right = np.asarray(right, dtype=np.float32)
    Wq = np.asarray(Wq, dtype=np.float32)
    Wkv = np.asarray(Wkv, dtype=np.float32)
    Wout = np.asarray(Wout, dtype=np.float32)
    bout = np.asarray(bout, dtype=np.float32)

    # 1/sqrt(DH) and the Schraudolph log2-scale are folded into Wq
    Wqs = (Wq * (scale * SCHR_A)).astype(BF16)  # (DQ, D)
    Wk = Wkv[:, :D].astype(BF16)               # (DC, D)
    Wv = Wkv[:, D:].astype(BF16)               # (DC, D)
    WoutB = Wout.astype(BF16)                  # (D, DQ)

    leftT = np.ascontiguousarray(left.transpose(0, 2, 1)).astype(BF16)    # (B, DQ, M)
    rightT = np.ascontiguousarray(right.transpose(0, 2, 1)).astype(BF16)  # (B, DC, N)
    maskT = np.ascontiguousarray(mask.transpose(0, 2, 1))                 # (B, N, M)
    mb = np.where(maskT, np.int16(int(SCHR_B)), np.int16(int(SCHR_BMASK)))
    ma = np.where(maskT, np.int16(0), np.int16(SCHR_MADD))
    m01 = maskT.astype(BF16)

    nc = _get_nc((M, N, DQmat, Hc, DH))

    in_maps = []
    for core in range(8):
        b, hh = divmod(core, 2)
        hs = slice(hh * Hc * DH, (hh + 1) * Hc * DH)
        in_maps.append(
            {
                "leftT": leftT[b],
                "rightT": rightT[b],
                "maskbias": mb[b],
                "maskadd": ma[b],
                "mask01": m01[b],
                "wq": np.ascontiguousarray(Wqs[:, hs]),
                "wk": np.ascontiguousarray(Wk[:, hs]),
                "wv": np.ascontiguousarray(Wv[:, hs]),
                "wout": np.ascontiguousarray(WoutB[hs, :]),
            }
        )

    tmpdir = None
    if TRACE:
        import shutil

        shutil.rmtree("/tmp/attn_trace", ignore_errors=True)
        tmpdir = "/tmp/attn_trace"
    res = run_bass_kernel_spmd(nc, in_maps, list(range(8)), trace=TRACE, tmpdir=tmpdir)
    LAST_RESULTS = res

    out = np.zeros((B, M, DQmat), np.float32)
    for core in range(8):
        out[core // 2] += res.results[core]["out_p"]
    out += bout[None, None, :]
    return out
